# revision 47
# baseline (speedup 1.0000x reference)
"""Trainium2 Bass kernel for nn_AttentionOnDetail (sparse_attention).

Computation (see reference): rms_norm -> qkv proj -> per-head rms_norm ->
rotary -> sigmoid-gated causal cumulative linear attention -> SwiGLU ->
out proj + residual -> relu^2.

Sharding (8 cores, one chip):
  Stage 1 (qkv + gated cumsum): core c handles batch b=c//2, head-half
  hh=c%2 (8 of 16 heads), ALL 8192 tokens -> the causal cumsum is fully
  core-local.  The y^T exchange only needs to happen inside each core
  pair (2b, 2b+1): they produce the two feature halves of batch b's y
  and also consume the two token halves in stage 2.  Every 512-token
  sub-chunk is exchanged with a pair-wise AllGather (16 small
  collectives, 1 MB output each) as soon as stage 1 produces it; the
  corresponding 512-token stage-2 chunk is interleaved into the stage-1
  tile stream a couple of tiles later, so the PE never drains.  Stage-2
  token ownership is interleaved at 512-token granularity (core parity
  par owns sub-chunks sc%2==par), which makes chunk j of both pair
  members ready after sub-chunk 2j+1's exchange -- the single SPMD
  program stays uniform, with per-core read offsets supplied as
  register-loaded row indices.

Algebraic restructurings (validated vs reference in fp32):
  - rms_norm(x) cancels inside the per-head rms_norm of q,k:
    rms_norm(r*z) = z * rsqrt(mean z^2 + eps/r^2); the qkv matmul
    consumes RAW x^T and only the v path needs the r_t scale.
  - r_t is folded into the causal-cumsum triangular matmul:
    S^T = s^T-blocks @ (U * r), computed on the PE; this also yields
    the f-major transpose stage 2 needs for free.
  - rotary tables have 16 real freqs + 16 zeros: only d in [0,16) and
    [32,48) of every 64-d head actually rotate (done in-place, 6 ops).
  - rsqrt runs on the vector engine (bit-hack + Newton steps) so the
    scalar engine's activation-table stays on Sigmoid/Silu/Square.

Stage 1 runs in [token-partition, feature-free] orientation, stage 2 in
[feature-partition, token-free]; the host supplies pre-transposed and
pre-tiled operands so the only on-device layout change is the PE-fused
cumsum-transpose.
"""

import sys
from contextlib import ExitStack

if "/opt/trn_rl_repo" not in sys.path:
    sys.path.insert(0, "/opt/trn_rl_repo")

import numpy as np
import ml_dtypes

import concourse.bacc as bacc
import concourse.bass as bass
import concourse.tile as tile
from concourse import mybir
from concourse.bass_utils import run_bass_kernel_spmd


def _install_neff_disk_cache():
    """Cache walrus NEFF compiles on disk, keyed by BIR hash."""
    import hashlib
    import os

    import concourse.bass2jax as b2j

    if getattr(b2j, "_neff_disk_cache_installed", False):
        return
    cache_dir = os.path.join(os.path.expanduser("~"), ".bass_neff_cache")
    os.makedirs(cache_dir, exist_ok=True)
    orig = b2j.compile_bir_kernel

    def cached(bir_json, tmpdir, neff_name="file.neff"):
        key = hashlib.sha256(bir_json).hexdigest()[:32]
        path = os.path.join(cache_dir, key + ".neff")
        dst = os.path.join(tmpdir, neff_name)
        if os.path.exists(path):
            with open(path, "rb") as f:
                data = f.read()
            with open(dst, "wb") as f:
                f.write(data)
            return dst
        out = orig(bir_json, tmpdir, neff_name=neff_name)
        tmp = path + ".tmp"
        with open(out, "rb") as f:
            data = f.read()
        with open(tmp, "wb") as f:
            f.write(data)
        os.replace(tmp, path)
        return out

    b2j.compile_bir_kernel = cached
    b2j._neff_disk_cache_installed = True

    import concourse.bass_utils as bu
    _orig_args = bu.get_walrus_args

    def _args(arch, tmpdir, **kw):
        a = _orig_args(arch, tmpdir, **kw)
        return [x.replace("--enable-ldw-opt=false", "--enable-ldw-opt=true")
                for x in a]

    bu.get_walrus_args = _args


_install_neff_disk_cache()

P = 128
C = 1024          # n_embd == n_qkv
NHL = 8           # heads per core (local)
DH = 64
FQKV = 3 * NHL * DH   # 1536 local qkv features
NCORES = 8

f32 = mybir.dt.float32
bf16 = mybir.dt.bfloat16
i32 = mybir.dt.int32
AF = mybir.ActivationFunctionType
ALU = mybir.AluOpType


def build(T):
    """Build the SPMD Bass program for total sequence length T."""
    NT = T // P             # stage-1 128-token tiles (64)
    NSC = T // 512          # 512-token sub-chunks (16)
    NCH = NSC // 2          # stage-2 chunks per core (8)
    TPC = NCH * 512         # stage-2 tokens per core (4096)

    nc = bacc.Bacc("TRN2", target_bir_lowering=False, debug=False,
                   num_devices=NCORES)

    # ---- per-core external inputs (host pre-tiled, partition-first) ----
    xt_blk = nc.dram_tensor("xt_blk", [NT // 4, P, 8, 512], bf16,
                            kind="ExternalInput")
    wqkvT = nc.dram_tensor("wqkvT", [P, 8, FQKV], bf16, kind="ExternalInput")
    wswiT = nc.dram_tensor("wswiT", [P, 8, 2 * C], bf16, kind="ExternalInput")
    woutT = nc.dram_tensor("woutT", [P, 8, C], bf16, kind="ExternalInput")
    xtres = nc.dram_tensor("xtres", [C, TPC], bf16, kind="ExternalInput")
    ptrig = nc.dram_tensor("ptrig", [P, NT, 32], bf16, kind="ExternalInput")
    ptrig0 = nc.dram_tensor("ptrig0", [P, 8, 32], bf16, kind="ExternalInput")
    pre = nc.dram_tensor("pre", [P, NT, 2], f32, kind="ExternalInput")
    utri = nc.dram_tensor("utri", [P, P], bf16, kind="ExternalInput")
    ident = nc.dram_tensor("ident", [P, P], bf16, kind="ExternalInput")
    yblk = nc.dram_tensor("yblk", [1, 24], i32, kind="ExternalInput")
    outT = nc.dram_tensor("outT", [C, TPC], bf16, kind="ExternalOutput")

    # ---- intermediates ----
    yhq = nc.dram_tensor("yhq", [NSC, 4, P, 512], bf16)
    agq = nc.dram_tensor("agq", [NSC * 2 * 4 * P, 512], bf16)
    # last two sub-chunks exchange in contiguous 256-token halves
    yhq2 = nc.dram_tensor("yhq2", [4, 4, P, 256], bf16)
    agq2 = nc.dram_tensor("agq2", [4 * 2 * 4 * P, 256], bf16)

    with tile.TileContext(nc) as tc, ExitStack() as ctx:
        consts = ctx.enter_context(tc.tile_pool(name="consts", bufs=1))
        s1 = ctx.enter_context(tc.tile_pool(name="s1", bufs=2))
        s1b = ctx.enter_context(tc.tile_pool(name="s1b", bufs=2))
        stg = ctx.enter_context(tc.tile_pool(name="stg", bufs=2))
        s2 = ctx.enter_context(tc.tile_pool(name="s2", bufs=2))
        s2c = ctx.enter_context(tc.tile_pool(name="s2c", bufs=1))
        mm = ctx.enter_context(tc.tile_pool(name="mm", bufs=4, space="PSUM"))
        mm2 = ctx.enter_context(tc.tile_pool(name="mm2", bufs=2, space="PSUM"))
        pS = ctx.enter_context(tc.tile_pool(name="pS", bufs=1, space="PSUM"))
        pQ = ctx.enter_context(tc.tile_pool(name="pQ", bufs=1, space="PSUM"))

        # ---- resident constants (spread across queues + split so the
        # first qkv matmuls only wait on yblk + wqA + xtA[0]; the big
        # stage-2 weights are issued from inside the loop) ----
        yblk_sb = consts.tile([1, 24], i32, tag="yblk")
        nc.sync.dma_start(out=yblk_sb, in_=yblk[:, :])
        wqt = []
        for g in range(4):
            w2 = consts.tile([P, 2, FQKV], bf16, tag=f"wq{g}")
            wqt.append(w2)
        nc.sync.dma_start(out=wqt[0], in_=wqkvT[:, 0:2, :])
        wsw_sb = consts.tile([P, 8, 2 * C], bf16, tag="wsw")
        wo_sb = consts.tile([P, 8, C], bf16, tag="wo")
        pre_sb = consts.tile([P, NT, 2], f32, tag="pre")
        nc.scalar.dma_start(out=pre_sb, in_=pre[:, :, :])
        utri_sb = consts.tile([P, P], bf16, tag="utri")
        nc.scalar.dma_start(out=utri_sb, in_=utri[:, :])
        ident_sb = consts.tile([P, P], bf16, tag="ident")
        nc.scalar.dma_start(out=ident_sb, in_=ident[:, :])
        # tiny duplicate trig table for tiles 0-3 keeps the 2 MB full
        # table off the startup critical path (it loads during tile 0+)
        ptrig0_sb = consts.tile([P, 8, 32], bf16, tag="ptrig0")
        nc.scalar.dma_start(out=ptrig0_sb, in_=ptrig0[:, :, :])
        ptrig_sb = consts.tile([P, NT, 32], bf16, tag="ptrig")
        zeros_sb = consts.tile([P, 4], f32, tag="zeros")
        nc.vector.memset(zeros_sb, 0.0)

        _, yvals = nc.values_load_multi_w_load_instructions(
            yblk_sb[0:1, 0:24], engines=[mybir.EngineType.SP],
            skip_runtime_bounds_check=True)

        s2state = {}

        def stage2_prefetch(j, half=None, W=512):
            """Fetch the exchanged y^T tiles for a stage-2 chunk, one tile
            ahead of its first matmuls so the PE never waits on the DMA."""
            ysb = []
            for fj in range(8):
                yt = s2.tile([P, W], bf16, tag=f"y{fj}")
                if half is None:
                    nc.sync.dma_start(
                        out=yt, in_=agq[bass.ds(yvals[fj] + 2048 * j, P), :])
                else:
                    nc.sync.dma_start(
                        out=yt,
                        in_=agq2[bass.ds(yvals[8 + fj] + 1024 * half, P), :])
                ysb.append(yt)
            s2state["ysb"] = ysb
            s2state["h"] = []

        def stage2_part(j, part, co=0, W=512):
            """One third of a stage-2 chunk, interleaved into the tile
            stream to smooth PE/scalar/DVE bursts.

            part 0: y fetch + swiglu qt 0-2; part 1: qt 3-5;
            part 2: qt 6-7 + out projection / residual / relu^2.
            silu(g)*u is computed as (u*g) * sigmoid(g) so the scalar
            engine never leaves the sigmoid/copy/identity/square table.
            """
            c0 = j * 512 + co
            ysb = s2state["ysb"]
            h_bf = s2state["h"]

            qts = range(3 * part, min(3 * part + 3, 8))
            for qt in qts:
                psu = mm2.tile([P, W], f32, tag="mm2")
                for fj in range(8):
                    nc.tensor.matmul(psu, lhsT=wsw_sb[:, fj, qt * P:(qt + 1) * P],
                                     rhs=ysb[fj], start=(fj == 0), stop=(fj == 7))
                psg = mm2.tile([P, W], f32, tag="mm2")
                for fj in range(8):
                    nc.tensor.matmul(psg,
                                     lhsT=wsw_sb[:, fj, (8 + qt) * P:(9 + qt) * P],
                                     rhs=ysb[fj], start=(fj == 0), stop=(fj == 7))
                us = s2c.tile([P, W], bf16, tag=f"us{qt}")
                if qt >= 6:
                    nc.vector.tensor_copy(us, psu)
                else:
                    nc.scalar.activation(out=us, in_=psu, func=AF.Copy)
                sg = s2c.tile([P, W], bf16, tag=f"g{qt}")
                nc.scalar.activation(out=sg, in_=psg, func=AF.Sigmoid)
                gu = s2c.tile([P, W], bf16, tag=f"gu{qt}")
                nc.vector.tensor_tensor(out=gu, in0=us, in1=psg, op=ALU.mult)
                ub = s2c.tile([P, W], bf16, tag=f"u{qt}")
                nc.vector.tensor_tensor(out=ub, in0=gu, in1=sg, op=ALU.mult)
                h_bf.append(ub)

            if part != 2:
                return
            for ctile in range(8):
                ps = mm2.tile([P, W], f32, tag="mm2")
                for qt in range(8):
                    nc.tensor.matmul(ps, lhsT=wo_sb[:, qt, ctile * P:(ctile + 1) * P],
                                     rhs=h_bf[qt], start=(qt == 0), stop=(qt == 7))
                xr = s2.tile([P, W], bf16, tag="xr")
                nc.sync.dma_start(out=xr,
                                  in_=xtres[ctile * P:(ctile + 1) * P, c0:c0 + W])
                res = s2.tile([P, W], f32, tag="res")
                nc.vector.tensor_tensor(out=res, in0=ps, in1=xr, op=ALU.add)
                nc.vector.tensor_scalar_max(out=res, in0=res, scalar1=0.0)
                sqo = s2.tile([P, W], bf16, tag="sqo")
                nc.scalar.activation(out=sqo, in_=res, func=AF.Square)
                nc.scalar.dma_start(
                    out=outT[ctile * P:(ctile + 1) * P, c0:c0 + W], in_=sqo)

        # ================= STAGE 1 (+ interleaved stage 2) ================
        # Each tile's cumsum/transpose PE work (which depends on a long
        # DVE chain) is deferred until after the NEXT tile's qkv matmuls,
        # so the PE never head-of-line blocks on the DVE chain.
        PAIRS = [[0, 1], [2, 3], [4, 5], [6, 7]]
        prev_yts = [None]  # previous tile's S^T+carry tile (carry source)
        tiles = {}         # i -> (s_bf, upr, sgk, ystage)
        xtA = xtB = ystage = None

        def tile_tail(it):
            """Cumsum + transpose + y^T staging for tile `it`."""
            s_bf, upr, sgk, ystg = tiles.pop(it)
            cbt = it % 4
            psS = pS.tile([P, 512], f32, tag="pS")
            for fj in range(4):
                nc.tensor.matmul(psS[:, fj * P:(fj + 1) * P],
                                 lhsT=s_bf[:, fj * P:(fj + 1) * P], rhs=upr,
                                 start=True, stop=True)
            # add running carry while evacuating (ACT Identity + col bias)
            yts = s1b.tile([P, 512], f32, tag="yts")
            for fj in range(4):
                if prev_yts[0] is None:
                    carry = zeros_sb[:, fj:fj + 1]
                else:
                    carry = prev_yts[0][:, fj * P + P - 1: fj * P + P]
                nc.scalar.activation(out=yts[:, fj * P:(fj + 1) * P],
                                     in_=psS[:, fj * P:(fj + 1) * P],
                                     func=AF.Identity, bias=carry, scale=1.0)
            prev_yts[0] = yts
            # sigma(q)^T (PE transpose) then y^T = sigma(q)^T * S^T
            psQ = pQ.tile([P, 512], bf16, tag="pQ")
            for fj in range(4):
                nc.tensor.matmul(psQ[:, fj * P:(fj + 1) * P],
                                 lhsT=sgk[:, fj * P:(fj + 1) * P], rhs=ident_sb,
                                 is_transpose=True, start=True, stop=True)
            nc.vector.tensor_tensor(
                out=ystg[:, :, cbt * P:(cbt + 1) * P],
                in0=psQ.rearrange("p (f t) -> p f t", t=P),
                in1=yts.rearrange("p (f t) -> p f t", t=P), op=ALU.mult)

            # flush + pair-exchange completed sub-chunks; the last two
            # exchange in contiguous 256-token halves so the final stage-2
            # chunk can be split and the serial tail shrinks.
            sct = it // 4
            if sct >= NSC - 2:
                if cbt in (1, 3):
                    h = cbt // 2
                    k = (sct - (NSC - 2)) * 2 + h
                    nc.gpsimd.dma_start(
                        out=yhq2[k, :, :, :].rearrange("f p t -> p f t"),
                        in_=ystg[:, :, h * 256:(h + 1) * 256])
                    nc.gpsimd.collective_compute(
                        "AllGather", ALU.bypass, replica_groups=PAIRS,
                        ins=[yhq2[k, :, :, :]],
                        outs=[agq2[k * 1024:(k + 1) * 1024, :]])
            elif cbt == 3:
                nc.gpsimd.dma_start(
                    out=yhq[sct, :, :, :].rearrange("f p t -> p f t"),
                    in_=ystg)
                nc.gpsimd.collective_compute(
                    "AllGather", ALU.bypass, replica_groups=PAIRS,
                    ins=[yhq[sct, :, :, :]],
                    outs=[agq[sct * 1024:(sct + 1) * 1024, :]])

        for i in range(NT + 1):
            if i == 1:
                # big tables / stage-2 weights: issued on the scalar queue
                # BEHIND tile 0's activations, so their transfers start
                # only after the startup-critical loads have the engines
                nc.scalar.dma_start(out=ptrig_sb, in_=ptrig[:, :, :])
                nc.scalar.dma_start(out=wsw_sb, in_=wswiT[:, :, :])
                nc.scalar.dma_start(out=wo_sb, in_=woutT[:, :, :])
            if i < NT:
                sc, cb = i // 4, i % 4
                if i == 0:
                    xtA = s1.tile([P, 4, 512], bf16, tag="xtA")
                    nc.sync.dma_start(out=xtA, in_=xt_blk[0, :, 0:4, :])
                    nc.sync.dma_start(out=wqt[1], in_=wqkvT[:, 2:4, :])
                    xtB = s1.tile([P, 4, 512], bf16, tag="xtB")
                    nc.sync.dma_start(out=xtB, in_=xt_blk[0, :, 4:8, :])
                    nc.sync.dma_start(out=wqt[2], in_=wqkvT[:, 4:6, :])
                    nc.sync.dma_start(out=wqt[3], in_=wqkvT[:, 6:8, :])
                if cb == 0:
                    if i > 0:
                        xtA, xtB = nxt
                    ystage = stg.tile([P, 4, 512], bf16, tag="ystage")
                    if i + 4 < NT:
                        # prefetch the next 4-tile group's x, a group ahead
                        nA = s1.tile([P, 4, 512], bf16, tag="xtA")
                        nc.sync.dma_start(out=nA,
                                          in_=xt_blk[i // 4 + 1, :, 0:4, :])
                        nB = s1.tile([P, 4, 512], bf16, tag="xtB")
                        nc.sync.dma_start(out=nB,
                                          in_=xt_blk[i // 4 + 1, :, 4:8, :])
                        nxt = (nA, nB)
                tsl = slice(cb * P, cb * P + P)

                # qkv projection: z[t, f] for f-chunks q|k|v (each 512).
                # j outer / fc inner: 3 consecutive matmuls share the
                # stationary xt tile so walrus ldw-opt elides LDWEIGHTS.
                zps = [mm.tile([P, 512], f32, tag="mm", name=f"z{_fc}_{i}")
                       for _fc in range(3)]
                for j in range(8):
                    xts = xtA if j < 4 else xtB
                    wqs = wqt[j // 2]
                    for fc in range(3):
                        nc.tensor.matmul(zps[fc], lhsT=xts[:, j % 4, tsl],
                                         rhs=wqs[:, j % 2, fc * 512:(fc + 1) * 512],
                                         start=(j == 0), stop=(j == 7))
                zq, zk, zv = zps
                upr = s1.tile([P, P], bf16, tag="upr")
                nc.vector.tensor_scalar_mul(out=upr, in0=utri_sb,
                                            scalar1=pre_sb[:, i, 0:1])

                # evacuate z to SBUF in bf16 (scalar), square on DVE (4x)
                zsb = s1.tile([P, 3, 512], bf16, tag="zsb")
                nc.scalar.activation(out=zsb[:, 0, :], in_=zq, func=AF.Copy)
                nc.scalar.activation(out=zsb[:, 1, :], in_=zk, func=AF.Copy)
                nc.scalar.activation(out=zsb[:, 2, :], in_=zv, func=AF.Copy)
                sq = s1.tile([P, 1024], bf16, tag="sq")
                nc.vector.tensor_tensor(out=sq[:, 0:512], in0=zsb[:, 0, :],
                                        in1=zsb[:, 0, :], op=ALU.mult)
                nc.vector.tensor_tensor(out=sq[:, 512:1024], in0=zsb[:, 1, :],
                                        in1=zsb[:, 1, :], op=ALU.mult)
                ss = s1.tile([P, 16], f32, tag="ss")
                nc.vector.tensor_reduce(
                    out=ss, in_=sq.rearrange("p (g d) -> p g d", d=DH),
                    axis=mybir.AxisListType.X, op=ALU.add)
                # ss = mean + eps
                nc.vector.tensor_scalar(out=ss, in0=ss, scalar1=1.0 / DH,
                                        scalar2=pre_sb[:, i, 1:2], op0=ALU.mult,
                                        op1=ALU.add)
                rr = s1.tile([P, 16], f32, tag="rr")
                tnw = s1.tile([P, 16], f32, tag="tnw")
                nc.vector.tensor_scalar(out=rr.bitcast(i32), in0=ss.bitcast(i32),
                                        scalar1=1, scalar2=None,
                                        op0=ALU.logical_shift_right)
                nc.vector.tensor_scalar(out=rr.bitcast(i32), in0=rr.bitcast(i32),
                                        scalar1=0x5F3759DF, scalar2=-1,
                                        op0=ALU.subtract, op1=ALU.mult)
                nc.vector.tensor_tensor(out=tnw, in0=rr, in1=rr, op=ALU.mult)
                nc.vector.tensor_tensor(out=tnw, in0=tnw, in1=ss, op=ALU.mult)
                nc.vector.tensor_scalar(out=tnw, in0=tnw, scalar1=-0.5,
                                        scalar2=1.5, op0=ALU.mult, op1=ALU.add)
                nc.vector.tensor_tensor(out=rr, in0=rr, in1=tnw, op=ALU.mult)

                qk = s1.tile([P, 16, DH], bf16, tag="qk")
                rq = rr[:, 0:8]
                rk = rr[:, 8:16]
                rr_q = bass.AP(tensor=rq.tensor, offset=rq.offset,
                               ap=[rq.ap[0], rq.ap[1], [0, DH]])
                rr_k = bass.AP(tensor=rk.tensor, offset=rk.offset,
                               ap=[rk.ap[0], rk.ap[1], [0, DH]])
                nc.vector.tensor_tensor(
                    out=qk[:, 0:8, :], in0=zsb[:, 0, :].rearrange(
                        "p (g d) -> p g d", d=DH), in1=rr_q, op=ALU.mult)
                nc.vector.tensor_tensor(
                    out=qk[:, 8:16, :], in0=zsb[:, 1, :].rearrange(
                        "p (g d) -> p g d", d=DH), in1=rr_k, op=ALU.mult)

                # rotary, in place: A1' = A1*c + A2*s ; A2' = A2*c - A1*s
                trig = ptrig0_sb if i < 8 else ptrig_sb
                ct = trig[:, i, 0:16]
                st = trig[:, i, 16:32]
                cb_ap = bass.AP(tensor=trig.tensor, offset=ct.offset,
                                ap=[ct.ap[0], [0, 16], [1, 16]])
                sb_ap = bass.AP(tensor=trig.tensor, offset=st.offset,
                                ap=[st.ap[0], [0, 16], [1, 16]])
                A1 = qk[:, :, 0:16]
                A2 = qk[:, :, 32:48]
                t1 = s1.tile([P, 16, 16], bf16, tag="t1")
                t2 = s1.tile([P, 16, 16], bf16, tag="t2")
                nc.vector.tensor_tensor(out=t1, in0=A1, in1=sb_ap, op=ALU.mult)
                nc.vector.tensor_tensor(out=A1, in0=A1, in1=cb_ap, op=ALU.mult)
                nc.vector.tensor_tensor(out=t2, in0=A2, in1=sb_ap, op=ALU.mult)
                nc.vector.tensor_tensor(out=A1, in0=A1, in1=t2, op=ALU.add)
                nc.vector.tensor_tensor(out=A2, in0=A2, in1=cb_ap, op=ALU.mult)
                nc.vector.tensor_tensor(out=A2, in0=A2, in1=t1, op=ALU.subtract)

                # gates: one sigmoid over q and k halves together
                sgk = s1.tile([P, 1024], bf16, tag="sgk")
                nc.scalar.activation(out=sgk,
                                     in_=qk.rearrange("p g d -> p (g d)"),
                                     func=AF.Sigmoid)
                s_bf = s1.tile([P, 512], bf16, tag="s_bf")
                nc.vector.tensor_tensor(out=s_bf, in0=sgk[:, 512:1024],
                                        in1=zsb[:, 2, :], op=ALU.mult)
                tiles[i] = (s_bf, upr, sgk, ystage)

            if i >= 1:
                tile_tail(i - 1)

            # stage-2: chunk j's y is prefetched at tile 8j+9 (its
            # pair-exchange completes during tile 8j+8) and its three
            # compute parts run at tiles 8j+10 / +12 / +14.
            if 8 <= i <= 56 and (i - 8) % 8 == 0:
                stage2_prefetch((i - 8) // 8)
            if 10 <= i <= 62 and (i - 10) % 2 == 0 and (i - 10) % 8 < 6:
                stage2_part((i - 10) // 8, ((i - 10) % 8) // 2)
            if i == NT - 2:
                stage2_prefetch(NCH - 1, half=0, W=256)
            if i == NT - 1:
                for p in range(3):
                    stage2_part(NCH - 1, p, co=0, W=256)
        stage2_prefetch(NCH - 1, half=1, W=256)
        for p in range(3):
            stage2_part(NCH - 1, p, co=256, W=256)

    nc.compile()
    return nc


_NC_CACHE = {}


def _get_nc(T):
    if T not in _NC_CACHE:
        _NC_CACHE[T] = build(T)
    return _NC_CACHE[T]


def host_prep(x, w_qkv, w_swiglu, w_out, T):
    """Build the 8 per-core input maps."""
    NT = T // P
    NCH = T // 1024
    bfd = ml_dtypes.bfloat16

    m2 = (x.astype(np.float64) ** 2).mean(-1).astype(np.float32)   # (B,T)
    r = (1.0 / np.sqrt(m2 + 1e-6)).astype(np.float32)
    epsq = (1e-6 * (m2 + 1e-6)).astype(np.float32)
    af = (1.0 / 1024.0) ** np.linspace(0, 1, 16, dtype=np.float32)
    th = np.arange(T, dtype=np.float32)[:, None] * af[None, :]
    cos16 = np.cos(th).astype(np.float32)
    sin16 = np.sin(th).astype(np.float32)
    utri_np = np.triu(np.ones((P, P))).astype(bfd)
    ident_np = np.eye(P).astype(bfd)

    wswiT_blk = np.ascontiguousarray(
        w_swiglu.T.reshape(8, P, 2 * C).transpose(1, 0, 2).astype(bfd))
    woutT_blk = np.ascontiguousarray(
        w_out.T.reshape(8, P, C).transpose(1, 0, 2).astype(bfd))

    in_maps = []
    for c in range(NCORES):
        b, par = c // 2, c % 2
        hh = par
        xb = x[b]                              # (T, C)
        # xt_blk[blk, p, j, t] = x[b][blk*512 + t, j*128 + p]
        xt_blk = np.ascontiguousarray(
            xb.reshape(NT // 4, 512, 8, P).transpose(0, 3, 2, 1).astype(bfd))
        rows = np.arange(512 * hh, 512 * hh + 512)
        wloc = np.concatenate(
            [w_qkv[rows], w_qkv[1024 + rows], w_qkv[2048 + rows]], 0)  # (1536, C)
        wqkvT_blk = np.ascontiguousarray(
            wloc.T.reshape(8, P, FQKV).transpose(1, 0, 2).astype(bfd))
        # stage-2 residual: core owns sub-chunks 2j+par, j=0..7
        xT = xb.T                              # (C, T)
        xtres_np = np.ascontiguousarray(np.concatenate(
            [xT[:, (2 * j + par) * 512:(2 * j + par) * 512 + 512]
             for j in range(NCH)], axis=1).astype(bfd))
        # packed tables, partition-first: cos16 | sin16 (bf16), r | eps (f32)
        tt = np.arange(T).reshape(NT, P)
        ptrig_np = np.zeros((NT, P, 32), np.float32)
        ptrig_np[:, :, 0:16] = cos16[tt]
        ptrig_np[:, :, 16:32] = sin16[tt]
        ptrig_np = np.ascontiguousarray(ptrig_np.transpose(1, 0, 2)).astype(bfd)
        pre_np = np.zeros((NT, P, 2), np.float32)
        pre_np[:, :, 0] = r[b][tt]
        pre_np[:, :, 1] = epsq[b][tt]
        pre_np = np.ascontiguousarray(pre_np.transpose(1, 0, 2))
        # agq row bases: sub-chunk block rows [s*1024, +1024) hold
        # [rank0 | rank1], each [4 fj, 128]; this core reads rank fj//4's
        # feature block fj%4 of ITS sub-chunk (s = 2j+par).
        yb = np.zeros(24, np.int32)
        for fj in range(8):
            yb[fj] = par * 1024 + (fj // 4) * 512 + (fj % 4) * P
            # half-exchange blocks: k = par*2 + h, row = k*1024 + rank*512
            # + (fj%4)*128 (the h*1024 term is added at compile time)
            yb[8 + fj] = par * 2048 + (fj // 4) * 512 + (fj % 4) * P
        in_maps.append({
            "xt_blk": xt_blk,
            "wqkvT": wqkvT_blk,
            "wswiT": wswiT_blk,
            "woutT": woutT_blk,
            "xtres": xtres_np,
            "ptrig": ptrig_np,
            "ptrig0": np.ascontiguousarray(ptrig_np[:, 0:8, :]),
            "pre": pre_np,
            "utri": utri_np,
            "ident": ident_np,
            "yblk": yb[None, :],
        })
    return in_maps


def assemble(results, B, T):
    out = np.zeros((B, T, C), np.float32)
    NCH = T // 1024
    for c in range(NCORES):
        b, par = c // 2, c % 2
        oT = results[c]["outT"].astype(np.float32)  # (C, TPC) bf16
        for j in range(NCH):
            t0 = (2 * j + par) * 512
            out[b, t0:t0 + 512, :] = oT[:, j * 512:(j + 1) * 512].T
    return out


def kernel(x, w_qkv, w_swiglu, w_out, n_head):
    x = np.asarray(x, dtype=np.float32)
    w_qkv = np.asarray(w_qkv, dtype=np.float32)
    w_swiglu = np.asarray(w_swiglu, dtype=np.float32)
    w_out = np.asarray(w_out, dtype=np.float32)
    B, T, _ = x.shape
    nc = _get_nc(T)
    in_maps = host_prep(x, w_qkv, w_swiglu, w_out, T)
    res = run_bass_kernel_spmd(nc, in_maps, list(range(NCORES)))
    return assemble(res.results, B, T)


# revision 49
# speedup vs baseline: 1.1734x; 1.1734x over previous
"""Trainium2 Bass kernel for nn_AttentionOnDetail (sparse_attention).

Computation (see reference): rms_norm -> qkv proj -> per-head rms_norm ->
rotary -> sigmoid-gated causal cumulative linear attention -> SwiGLU ->
out proj + residual -> relu^2.

Sharding (8 cores, one chip):
  Stage 1 (qkv + gated cumsum): core c handles batch b=c//2, head-half
  hh=c%2 (8 of 16 heads), ALL 8192 tokens -> the causal cumsum is fully
  core-local.  The y^T exchange only needs to happen inside each core
  pair (2b, 2b+1): they produce the two feature halves of batch b's y
  and also consume the two token halves in stage 2.  Every 512-token
  sub-chunk is exchanged with a pair-wise AllGather (1 MB output) as
  soon as stage 1 produces it (the last two sub-chunks in 256-token
  halves, shrinking the serial tail); the corresponding stage-2 chunk
  is interleaved into the stage-1 tile stream in three parts, a few
  tiles later, so the PE never drains.  Stage-2 token ownership is
  interleaved at 512-token granularity (core parity par owns sub-chunks
  sc%2==par), which makes chunk j of both pair members ready after
  sub-chunk 2j+1's exchange -- the single SPMD program stays uniform,
  with per-core read offsets supplied as register-loaded row indices.

Scheduling notes (from perfetto traces):
  - Each tile's cumsum/transpose PE work depends on a long DVE chain;
    it is deferred until after the NEXT tile's qkv matmuls so the PE
    (the critical engine, ~88% busy) never head-of-line blocks.
  - y^T sub-chunk flushes ride the gpsimd queue (ordered just before
    their collectives) and outT stores ride the scalar queue, so the
    final exchange is never FIFO-blocked behind stage-2 output DMAs.
  - Startup loads are split/ordered (yblk, wq j-pairs interleaved with
    x halves; big tables+stage-2 weights issued behind tile-0 acts) to
    fit the ~170 GB/s early DMA-engine bandwidth; a tiny duplicate
    trig table covers tiles 0-7 while the full one loads.
  - silu(g)*u is computed as (u*g)*sigmoid(g) so the scalar engine
    never leaves the sigmoid/copy/identity/square activation table
    (Silu lives in a different table; swapping cost 49 table loads).

Algebraic restructurings (validated vs reference in fp32):
  - rms_norm(x) cancels inside the per-head rms_norm of q,k:
    rms_norm(r*z) = z * rsqrt(mean z^2 + eps/r^2); the qkv matmul
    consumes RAW x^T and only the v path needs the r_t scale.
  - r_t is folded into the causal-cumsum triangular matmul:
    S^T = s^T-blocks @ (U * r), computed on the PE; this also yields
    the f-major transpose stage 2 needs for free.
  - rotary tables have 16 real freqs + 16 zeros: only d in [0,16) and
    [32,48) of every 64-d head actually rotate (done in-place, 6 ops).
  - rsqrt runs on the vector engine (bit-hack + Newton steps) so the
    scalar engine's activation-table stays on Sigmoid/Silu/Square.

Stage 1 runs in [token-partition, feature-free] orientation, stage 2 in
[feature-partition, token-free]; the host supplies pre-transposed and
pre-tiled operands so the only on-device layout change is the PE-fused
cumsum-transpose.
"""

import sys
from contextlib import ExitStack

if "/opt/trn_rl_repo" not in sys.path:
    sys.path.insert(0, "/opt/trn_rl_repo")

import numpy as np
import ml_dtypes

import concourse.bacc as bacc
import concourse.bass as bass
import concourse.tile as tile
from concourse import mybir
from concourse.bass_utils import run_bass_kernel_spmd


def _install_neff_disk_cache():
    """Cache walrus NEFF compiles on disk, keyed by BIR hash."""
    import hashlib
    import os

    import concourse.bass2jax as b2j

    if getattr(b2j, "_neff_disk_cache_installed", False):
        return
    cache_dir = os.path.join(os.path.expanduser("~"), ".bass_neff_cache")
    os.makedirs(cache_dir, exist_ok=True)
    orig = b2j.compile_bir_kernel

    def cached(bir_json, tmpdir, neff_name="file.neff"):
        key = hashlib.sha256(bir_json).hexdigest()[:32]
        path = os.path.join(cache_dir, key + ".neff")
        dst = os.path.join(tmpdir, neff_name)
        if os.path.exists(path):
            with open(path, "rb") as f:
                data = f.read()
            with open(dst, "wb") as f:
                f.write(data)
            return dst
        out = orig(bir_json, tmpdir, neff_name=neff_name)
        tmp = path + ".tmp"
        with open(out, "rb") as f:
            data = f.read()
        with open(tmp, "wb") as f:
            f.write(data)
        os.replace(tmp, path)
        return out

    b2j.compile_bir_kernel = cached
    b2j._neff_disk_cache_installed = True

    import concourse.bass_utils as bu
    _orig_args = bu.get_walrus_args

    def _args(arch, tmpdir, **kw):
        a = _orig_args(arch, tmpdir, **kw)
        return [x.replace("--enable-ldw-opt=false", "--enable-ldw-opt=true")
                for x in a]

    bu.get_walrus_args = _args


_install_neff_disk_cache()

P = 128
C = 1024          # n_embd == n_qkv
NHL = 8           # heads per core (local)
DH = 64
FQKV = 3 * NHL * DH   # 1536 local qkv features
NCORES = 8

f32 = mybir.dt.float32
bf16 = mybir.dt.bfloat16
i32 = mybir.dt.int32
AF = mybir.ActivationFunctionType
ALU = mybir.AluOpType


def build(T):
    """Build the SPMD Bass program for total sequence length T."""
    NT = T // P             # stage-1 128-token tiles (64)
    NSC = T // 512          # 512-token sub-chunks (16)
    NCH = NSC // 2          # stage-2 chunks per core (8)
    TPC = NCH * 512         # stage-2 tokens per core (4096)

    nc = bacc.Bacc("TRN2", target_bir_lowering=False, debug=False,
                   num_devices=NCORES)

    # ---- per-core external inputs (host pre-tiled, partition-first) ----
    xt_blk = nc.dram_tensor("xt_blk", [NT // 4, P, 8, 512], bf16,
                            kind="ExternalInput")
    wqkvT = nc.dram_tensor("wqkvT", [P, 8, FQKV], bf16, kind="ExternalInput")
    wswiT = nc.dram_tensor("wswiT", [P, 8, 2 * C], bf16, kind="ExternalInput")
    woutT = nc.dram_tensor("woutT", [P, 8, C], bf16, kind="ExternalInput")
    xtres = nc.dram_tensor("xtres", [C, TPC], bf16, kind="ExternalInput")
    ptrig = nc.dram_tensor("ptrig", [P, NT, 32], bf16, kind="ExternalInput")
    ptrig0 = nc.dram_tensor("ptrig0", [P, 8, 32], bf16, kind="ExternalInput")
    pre = nc.dram_tensor("pre", [P, NT, 2], f32, kind="ExternalInput")
    utri = nc.dram_tensor("utri", [P, P], bf16, kind="ExternalInput")
    ident = nc.dram_tensor("ident", [P, P], bf16, kind="ExternalInput")
    yblk = nc.dram_tensor("yblk", [1, 24], i32, kind="ExternalInput")
    outT = nc.dram_tensor("outT", [C, TPC], bf16, kind="ExternalOutput")

    # ---- intermediates ----
    yhq = nc.dram_tensor("yhq", [NSC, 4, P, 512], bf16)
    agq = nc.dram_tensor("agq", [NSC * 2 * 4 * P, 512], bf16)
    # last two sub-chunks exchange in contiguous 256-token halves
    yhq2 = nc.dram_tensor("yhq2", [4, 4, P, 256], bf16)
    agq2 = nc.dram_tensor("agq2", [4 * 2 * 4 * P, 256], bf16)

    with tile.TileContext(nc) as tc, ExitStack() as ctx:
        consts = ctx.enter_context(tc.tile_pool(name="consts", bufs=1))
        s1 = ctx.enter_context(tc.tile_pool(name="s1", bufs=2))
        s1b = ctx.enter_context(tc.tile_pool(name="s1b", bufs=2))
        stg = ctx.enter_context(tc.tile_pool(name="stg", bufs=2))
        s2 = ctx.enter_context(tc.tile_pool(name="s2", bufs=2))
        s2c = ctx.enter_context(tc.tile_pool(name="s2c", bufs=1))
        mm = ctx.enter_context(tc.tile_pool(name="mm", bufs=4, space="PSUM"))
        mm2 = ctx.enter_context(tc.tile_pool(name="mm2", bufs=2, space="PSUM"))
        pS = ctx.enter_context(tc.tile_pool(name="pS", bufs=1, space="PSUM"))
        pQ = ctx.enter_context(tc.tile_pool(name="pQ", bufs=1, space="PSUM"))

        # ---- resident constants (spread across queues + split so the
        # first qkv matmuls only wait on yblk + wqA + xtA[0]; the big
        # stage-2 weights are issued from inside the loop) ----
        yblk_sb = consts.tile([1, 24], i32, tag="yblk")
        nc.sync.dma_start(out=yblk_sb, in_=yblk[:, :])
        wqt = []
        for g in range(4):
            w2 = consts.tile([P, 2, FQKV], bf16, tag=f"wq{g}")
            wqt.append(w2)
        nc.sync.dma_start(out=wqt[0], in_=wqkvT[:, 0:2, :])
        wsw_sb = consts.tile([P, 8, 2 * C], bf16, tag="wsw")
        wo_sb = consts.tile([P, 8, C], bf16, tag="wo")
        pre_sb = consts.tile([P, NT, 2], f32, tag="pre")
        nc.scalar.dma_start(out=pre_sb, in_=pre[:, :, :])
        utri_sb = consts.tile([P, P], bf16, tag="utri")
        nc.scalar.dma_start(out=utri_sb, in_=utri[:, :])
        ident_sb = consts.tile([P, P], bf16, tag="ident")
        nc.scalar.dma_start(out=ident_sb, in_=ident[:, :])
        # tiny duplicate trig table for tiles 0-3 keeps the 2 MB full
        # table off the startup critical path (it loads during tile 0+)
        ptrig0_sb = consts.tile([P, 8, 32], bf16, tag="ptrig0")
        nc.scalar.dma_start(out=ptrig0_sb, in_=ptrig0[:, :, :])
        ptrig_sb = consts.tile([P, NT, 32], bf16, tag="ptrig")
        zeros_sb = consts.tile([P, 4], f32, tag="zeros")
        nc.vector.memset(zeros_sb, 0.0)

        _, yvals = nc.values_load_multi_w_load_instructions(
            yblk_sb[0:1, 0:24], engines=[mybir.EngineType.SP],
            skip_runtime_bounds_check=True)

        s2state = {}

        def stage2_prefetch(j, half=None, W=512):
            """Fetch the exchanged y^T tiles for a stage-2 chunk, one tile
            ahead of its first matmuls so the PE never waits on the DMA."""
            ysb = []
            for fj in range(8):
                yt = s2.tile([P, W], bf16, tag=f"y{fj}")
                if half is None:
                    nc.sync.dma_start(
                        out=yt, in_=agq[bass.ds(yvals[fj] + 2048 * j, P), :])
                else:
                    nc.sync.dma_start(
                        out=yt,
                        in_=agq2[bass.ds(yvals[8 + fj] + 1024 * half, P), :])
                ysb.append(yt)
            s2state["ysb"] = ysb
            s2state["h"] = []

        def stage2_part(j, part, co=0, W=512):
            """One third of a stage-2 chunk, interleaved into the tile
            stream to smooth PE/scalar/DVE bursts.

            part 0: y fetch + swiglu qt 0-2; part 1: qt 3-5;
            part 2: qt 6-7 + out projection / residual / relu^2.
            silu(g)*u is computed as (u*g) * sigmoid(g) so the scalar
            engine never leaves the sigmoid/copy/identity/square table.
            """
            c0 = j * 512 + co
            ysb = s2state["ysb"]
            h_bf = s2state["h"]

            qts = range(3 * part, min(3 * part + 3, 8))
            for qt in qts:
                psu = mm2.tile([P, W], f32, tag="mm2")
                for fj in range(8):
                    nc.tensor.matmul(psu, lhsT=wsw_sb[:, fj, qt * P:(qt + 1) * P],
                                     rhs=ysb[fj], start=(fj == 0), stop=(fj == 7))
                psg = mm2.tile([P, W], f32, tag="mm2")
                for fj in range(8):
                    nc.tensor.matmul(psg,
                                     lhsT=wsw_sb[:, fj, (8 + qt) * P:(9 + qt) * P],
                                     rhs=ysb[fj], start=(fj == 0), stop=(fj == 7))
                us = s2c.tile([P, W], bf16, tag=f"us{qt}")
                nc.scalar.activation(out=us, in_=psu, func=AF.Copy)
                sg = s2c.tile([P, W], bf16, tag=f"g{qt}")
                nc.scalar.activation(out=sg, in_=psg, func=AF.Sigmoid)
                gu = s2c.tile([P, W], bf16, tag=f"gu{qt}")
                nc.vector.tensor_tensor(out=gu, in0=us, in1=psg, op=ALU.mult)
                ub = s2c.tile([P, W], bf16, tag=f"u{qt}")
                nc.vector.tensor_tensor(out=ub, in0=gu, in1=sg, op=ALU.mult)
                h_bf.append(ub)

            if part != 2:
                return
            for ctile in range(8):
                ps = mm2.tile([P, W], f32, tag="mm2")
                for qt in range(8):
                    nc.tensor.matmul(ps, lhsT=wo_sb[:, qt, ctile * P:(ctile + 1) * P],
                                     rhs=h_bf[qt], start=(qt == 0), stop=(qt == 7))
                xr = s2.tile([P, W], bf16, tag="xr")
                nc.sync.dma_start(out=xr,
                                  in_=xtres[ctile * P:(ctile + 1) * P, c0:c0 + W])
                res = s2.tile([P, W], f32, tag="res")
                nc.vector.tensor_tensor(out=res, in0=ps, in1=xr, op=ALU.add)
                nc.vector.tensor_scalar_max(out=res, in0=res, scalar1=0.0)
                sqo = s2.tile([P, W], bf16, tag="sqo")
                nc.scalar.activation(out=sqo, in_=res, func=AF.Square)
                nc.scalar.dma_start(
                    out=outT[ctile * P:(ctile + 1) * P, c0:c0 + W], in_=sqo)

        # ================= STAGE 1 (+ interleaved stage 2) ================
        # Each tile's cumsum/transpose PE work (which depends on a long
        # DVE chain) is deferred until after the NEXT tile's qkv matmuls,
        # so the PE never head-of-line blocks on the DVE chain.
        PAIRS = [[0, 1], [2, 3], [4, 5], [6, 7]]
        prev_yts = [None]  # previous tile's S^T+carry tile (carry source)
        tiles = {}         # i -> (s_bf, upr, sgk, ystage)
        xtA = xtB = ystage = None

        def tile_tail(it):
            """Cumsum + transpose + y^T staging for tile `it`."""
            s_bf, upr, sgk, ystg = tiles.pop(it)
            cbt = it % 4
            psS = pS.tile([P, 512], f32, tag="pS")
            for fj in range(4):
                nc.tensor.matmul(psS[:, fj * P:(fj + 1) * P],
                                 lhsT=s_bf[:, fj * P:(fj + 1) * P], rhs=upr,
                                 start=True, stop=True)
            # add running carry while evacuating (ACT Identity + col bias)
            yts = s1b.tile([P, 512], f32, tag="yts")
            for fj in range(4):
                if prev_yts[0] is None:
                    carry = zeros_sb[:, fj:fj + 1]
                else:
                    carry = prev_yts[0][:, fj * P + P - 1: fj * P + P]
                nc.scalar.activation(out=yts[:, fj * P:(fj + 1) * P],
                                     in_=psS[:, fj * P:(fj + 1) * P],
                                     func=AF.Identity, bias=carry, scale=1.0)
            prev_yts[0] = yts
            # sigma(q)^T (PE transpose) then y^T = sigma(q)^T * S^T
            psQ = pQ.tile([P, 512], bf16, tag="pQ")
            for fj in range(4):
                nc.tensor.matmul(psQ[:, fj * P:(fj + 1) * P],
                                 lhsT=sgk[:, fj * P:(fj + 1) * P], rhs=ident_sb,
                                 is_transpose=True, start=True, stop=True)
            nc.vector.tensor_tensor(
                out=ystg[:, :, cbt * P:(cbt + 1) * P],
                in0=psQ.rearrange("p (f t) -> p f t", t=P),
                in1=yts.rearrange("p (f t) -> p f t", t=P), op=ALU.mult)

            # flush + pair-exchange completed sub-chunks; the last two
            # exchange in contiguous 256-token halves so the final stage-2
            # chunk can be split and the serial tail shrinks.
            sct = it // 4
            if sct >= NSC - 2:
                if cbt in (1, 3):
                    h = cbt // 2
                    k = (sct - (NSC - 2)) * 2 + h
                    nc.gpsimd.dma_start(
                        out=yhq2[k, :, :, :].rearrange("f p t -> p f t"),
                        in_=ystg[:, :, h * 256:(h + 1) * 256])
                    nc.gpsimd.collective_compute(
                        "AllGather", ALU.bypass, replica_groups=PAIRS,
                        ins=[yhq2[k, :, :, :]],
                        outs=[agq2[k * 1024:(k + 1) * 1024, :]])
            elif cbt == 3:
                nc.gpsimd.dma_start(
                    out=yhq[sct, :, :, :].rearrange("f p t -> p f t"),
                    in_=ystg)
                nc.gpsimd.collective_compute(
                    "AllGather", ALU.bypass, replica_groups=PAIRS,
                    ins=[yhq[sct, :, :, :]],
                    outs=[agq[sct * 1024:(sct + 1) * 1024, :]])

        for i in range(NT + 1):
            if i == 1:
                # big tables / stage-2 weights: issued on the scalar queue
                # BEHIND tile 0's activations, so their transfers start
                # only after the startup-critical loads have the engines
                nc.scalar.dma_start(out=ptrig_sb, in_=ptrig[:, :, :])
                nc.scalar.dma_start(out=wsw_sb, in_=wswiT[:, :, :])
                nc.scalar.dma_start(out=wo_sb, in_=woutT[:, :, :])
            if i < NT:
                sc, cb = i // 4, i % 4
                if i == 0:
                    xtA = s1.tile([P, 4, 512], bf16, tag="xtA")
                    nc.sync.dma_start(out=xtA, in_=xt_blk[0, :, 0:4, :])
                    nc.sync.dma_start(out=wqt[1], in_=wqkvT[:, 2:4, :])
                    xtB = s1.tile([P, 4, 512], bf16, tag="xtB")
                    nc.sync.dma_start(out=xtB, in_=xt_blk[0, :, 4:8, :])
                    nc.sync.dma_start(out=wqt[2], in_=wqkvT[:, 4:6, :])
                    nc.sync.dma_start(out=wqt[3], in_=wqkvT[:, 6:8, :])
                if cb == 0:
                    if i > 0:
                        xtA, xtB = nxt
                    ystage = stg.tile([P, 4, 512], bf16, tag="ystage")
                    if i + 4 < NT:
                        # prefetch the next 4-tile group's x, a group ahead
                        nA = s1.tile([P, 4, 512], bf16, tag="xtA")
                        nc.sync.dma_start(out=nA,
                                          in_=xt_blk[i // 4 + 1, :, 0:4, :])
                        nB = s1.tile([P, 4, 512], bf16, tag="xtB")
                        nc.sync.dma_start(out=nB,
                                          in_=xt_blk[i // 4 + 1, :, 4:8, :])
                        nxt = (nA, nB)
                tsl = slice(cb * P, cb * P + P)

                # qkv projection: z[t, f] for f-chunks q|k|v (each 512).
                # j outer / fc inner: 3 consecutive matmuls share the
                # stationary xt tile so walrus ldw-opt elides LDWEIGHTS.
                zps = [mm.tile([P, 512], f32, tag="mm", name=f"z{_fc}_{i}")
                       for _fc in range(3)]
                for j in range(8):
                    xts = xtA if j < 4 else xtB
                    wqs = wqt[j // 2]
                    for fc in range(3):
                        nc.tensor.matmul(zps[fc], lhsT=xts[:, j % 4, tsl],
                                         rhs=wqs[:, j % 2, fc * 512:(fc + 1) * 512],
                                         start=(j == 0), stop=(j == 7))
                zq, zk, zv = zps
                upr = s1.tile([P, P], bf16, tag="upr")
                nc.vector.tensor_scalar_mul(out=upr, in0=utri_sb,
                                            scalar1=pre_sb[:, i, 0:1])

                # evacuate z to SBUF in bf16 (scalar), square on DVE (4x)
                zsb = s1.tile([P, 3, 512], bf16, tag="zsb")
                nc.scalar.activation(out=zsb[:, 0, :], in_=zq, func=AF.Copy)
                nc.scalar.activation(out=zsb[:, 1, :], in_=zk, func=AF.Copy)
                nc.scalar.activation(out=zsb[:, 2, :], in_=zv, func=AF.Copy)
                sq = s1.tile([P, 1024], bf16, tag="sq")
                nc.vector.tensor_tensor(out=sq[:, 0:512], in0=zsb[:, 0, :],
                                        in1=zsb[:, 0, :], op=ALU.mult)
                nc.vector.tensor_tensor(out=sq[:, 512:1024], in0=zsb[:, 1, :],
                                        in1=zsb[:, 1, :], op=ALU.mult)
                ss = s1.tile([P, 16], f32, tag="ss")
                nc.vector.tensor_reduce(
                    out=ss, in_=sq.rearrange("p (g d) -> p g d", d=DH),
                    axis=mybir.AxisListType.X, op=ALU.add)
                # ss = mean + eps
                nc.vector.tensor_scalar(out=ss, in0=ss, scalar1=1.0 / DH,
                                        scalar2=pre_sb[:, i, 1:2], op0=ALU.mult,
                                        op1=ALU.add)
                rr = s1.tile([P, 16], f32, tag="rr")
                tnw = s1.tile([P, 16], f32, tag="tnw")
                nc.vector.tensor_scalar(out=rr.bitcast(i32), in0=ss.bitcast(i32),
                                        scalar1=1, scalar2=None,
                                        op0=ALU.logical_shift_right)
                nc.vector.tensor_scalar(out=rr.bitcast(i32), in0=rr.bitcast(i32),
                                        scalar1=0x5F3759DF, scalar2=-1,
                                        op0=ALU.subtract, op1=ALU.mult)
                nc.vector.tensor_tensor(out=tnw, in0=rr, in1=rr, op=ALU.mult)
                nc.vector.tensor_tensor(out=tnw, in0=tnw, in1=ss, op=ALU.mult)
                nc.vector.tensor_scalar(out=tnw, in0=tnw, scalar1=-0.5,
                                        scalar2=1.5, op0=ALU.mult, op1=ALU.add)
                nc.vector.tensor_tensor(out=rr, in0=rr, in1=tnw, op=ALU.mult)

                qk = s1.tile([P, 16, DH], bf16, tag="qk")
                rq = rr[:, 0:8]
                rk = rr[:, 8:16]
                rr_q = bass.AP(tensor=rq.tensor, offset=rq.offset,
                               ap=[rq.ap[0], rq.ap[1], [0, DH]])
                rr_k = bass.AP(tensor=rk.tensor, offset=rk.offset,
                               ap=[rk.ap[0], rk.ap[1], [0, DH]])
                nc.vector.tensor_tensor(
                    out=qk[:, 0:8, :], in0=zsb[:, 0, :].rearrange(
                        "p (g d) -> p g d", d=DH), in1=rr_q, op=ALU.mult)
                nc.vector.tensor_tensor(
                    out=qk[:, 8:16, :], in0=zsb[:, 1, :].rearrange(
                        "p (g d) -> p g d", d=DH), in1=rr_k, op=ALU.mult)

                # rotary, in place: A1' = A1*c + A2*s ; A2' = A2*c - A1*s
                trig = ptrig0_sb if i < 8 else ptrig_sb
                ct = trig[:, i, 0:16]
                st = trig[:, i, 16:32]
                cb_ap = bass.AP(tensor=trig.tensor, offset=ct.offset,
                                ap=[ct.ap[0], [0, 16], [1, 16]])
                sb_ap = bass.AP(tensor=trig.tensor, offset=st.offset,
                                ap=[st.ap[0], [0, 16], [1, 16]])
                A1 = qk[:, :, 0:16]
                A2 = qk[:, :, 32:48]
                t1 = s1.tile([P, 16, 16], bf16, tag="t1")
                t2 = s1.tile([P, 16, 16], bf16, tag="t2")
                nc.vector.tensor_tensor(out=t1, in0=A1, in1=sb_ap, op=ALU.mult)
                nc.vector.tensor_tensor(out=A1, in0=A1, in1=cb_ap, op=ALU.mult)
                nc.vector.tensor_tensor(out=t2, in0=A2, in1=sb_ap, op=ALU.mult)
                nc.vector.tensor_tensor(out=A1, in0=A1, in1=t2, op=ALU.add)
                nc.vector.tensor_tensor(out=A2, in0=A2, in1=cb_ap, op=ALU.mult)
                nc.vector.tensor_tensor(out=A2, in0=A2, in1=t1, op=ALU.subtract)

                # gates: one sigmoid over q and k halves together
                sgk = s1.tile([P, 1024], bf16, tag="sgk")
                nc.scalar.activation(out=sgk,
                                     in_=qk.rearrange("p g d -> p (g d)"),
                                     func=AF.Sigmoid)
                s_bf = s1.tile([P, 512], bf16, tag="s_bf")
                nc.vector.tensor_tensor(out=s_bf, in0=sgk[:, 512:1024],
                                        in1=zsb[:, 2, :], op=ALU.mult)
                tiles[i] = (s_bf, upr, sgk, ystage)

            if i >= 1:
                tile_tail(i - 1)

            # stage-2: chunk j's y is prefetched at tile 8j+9 (its
            # pair-exchange completes during tile 8j+8) and its three
            # compute parts run at tiles 8j+10 / +12 / +14.
            if 8 <= i <= 56 and (i - 8) % 8 == 0:
                stage2_prefetch((i - 8) // 8)
            if 10 <= i <= 62 and (i - 10) % 2 == 0 and (i - 10) % 8 < 6:
                stage2_part((i - 10) // 8, ((i - 10) % 8) // 2)
            if i == NT - 2:
                stage2_prefetch(NCH - 1, half=0, W=256)
            if i == NT - 1:
                for p in range(3):
                    stage2_part(NCH - 1, p, co=0, W=256)
        stage2_prefetch(NCH - 1, half=1, W=256)
        for p in range(3):
            stage2_part(NCH - 1, p, co=256, W=256)

    nc.compile()
    return nc


_NC_CACHE = {}


def _get_nc(T):
    if T not in _NC_CACHE:
        _NC_CACHE[T] = build(T)
    return _NC_CACHE[T]


def host_prep(x, w_qkv, w_swiglu, w_out, T):
    """Build the 8 per-core input maps."""
    NT = T // P
    NCH = T // 1024
    bfd = ml_dtypes.bfloat16

    m2 = (x.astype(np.float64) ** 2).mean(-1).astype(np.float32)   # (B,T)
    r = (1.0 / np.sqrt(m2 + 1e-6)).astype(np.float32)
    epsq = (1e-6 * (m2 + 1e-6)).astype(np.float32)
    af = (1.0 / 1024.0) ** np.linspace(0, 1, 16, dtype=np.float32)
    th = np.arange(T, dtype=np.float32)[:, None] * af[None, :]
    cos16 = np.cos(th).astype(np.float32)
    sin16 = np.sin(th).astype(np.float32)
    utri_np = np.triu(np.ones((P, P))).astype(bfd)
    ident_np = np.eye(P).astype(bfd)

    wswiT_blk = np.ascontiguousarray(
        w_swiglu.T.reshape(8, P, 2 * C).transpose(1, 0, 2).astype(bfd))
    woutT_blk = np.ascontiguousarray(
        w_out.T.reshape(8, P, C).transpose(1, 0, 2).astype(bfd))

    in_maps = []
    for c in range(NCORES):
        b, par = c // 2, c % 2
        hh = par
        xb = x[b]                              # (T, C)
        # xt_blk[blk, p, j, t] = x[b][blk*512 + t, j*128 + p]
        xt_blk = np.ascontiguousarray(
            xb.reshape(NT // 4, 512, 8, P).transpose(0, 3, 2, 1).astype(bfd))
        rows = np.arange(512 * hh, 512 * hh + 512)
        wloc = np.concatenate(
            [w_qkv[rows], w_qkv[1024 + rows], w_qkv[2048 + rows]], 0)  # (1536, C)
        wqkvT_blk = np.ascontiguousarray(
            wloc.T.reshape(8, P, FQKV).transpose(1, 0, 2).astype(bfd))
        # stage-2 residual: core owns sub-chunks 2j+par, j=0..7
        xT = xb.T                              # (C, T)
        xtres_np = np.ascontiguousarray(np.concatenate(
            [xT[:, (2 * j + par) * 512:(2 * j + par) * 512 + 512]
             for j in range(NCH)], axis=1).astype(bfd))
        # packed tables, partition-first: cos16 | sin16 (bf16), r | eps (f32)
        tt = np.arange(T).reshape(NT, P)
        ptrig_np = np.zeros((NT, P, 32), np.float32)
        ptrig_np[:, :, 0:16] = cos16[tt]
        ptrig_np[:, :, 16:32] = sin16[tt]
        ptrig_np = np.ascontiguousarray(ptrig_np.transpose(1, 0, 2)).astype(bfd)
        pre_np = np.zeros((NT, P, 2), np.float32)
        pre_np[:, :, 0] = r[b][tt]
        pre_np[:, :, 1] = epsq[b][tt]
        pre_np = np.ascontiguousarray(pre_np.transpose(1, 0, 2))
        # agq row bases: sub-chunk block rows [s*1024, +1024) hold
        # [rank0 | rank1], each [4 fj, 128]; this core reads rank fj//4's
        # feature block fj%4 of ITS sub-chunk (s = 2j+par).
        yb = np.zeros(24, np.int32)
        for fj in range(8):
            yb[fj] = par * 1024 + (fj // 4) * 512 + (fj % 4) * P
            # half-exchange blocks: k = par*2 + h, row = k*1024 + rank*512
            # + (fj%4)*128 (the h*1024 term is added at compile time)
            yb[8 + fj] = par * 2048 + (fj // 4) * 512 + (fj % 4) * P
        in_maps.append({
            "xt_blk": xt_blk,
            "wqkvT": wqkvT_blk,
            "wswiT": wswiT_blk,
            "woutT": woutT_blk,
            "xtres": xtres_np,
            "ptrig": ptrig_np,
            "ptrig0": np.ascontiguousarray(ptrig_np[:, 0:8, :]),
            "pre": pre_np,
            "utri": utri_np,
            "ident": ident_np,
            "yblk": yb[None, :],
        })
    return in_maps


def assemble(results, B, T):
    out = np.zeros((B, T, C), np.float32)
    NCH = T // 1024
    for c in range(NCORES):
        b, par = c // 2, c % 2
        oT = results[c]["outT"].astype(np.float32)  # (C, TPC) bf16
        for j in range(NCH):
            t0 = (2 * j + par) * 512
            out[b, t0:t0 + 512, :] = oT[:, j * 512:(j + 1) * 512].T
    return out


def kernel(x, w_qkv, w_swiglu, w_out, n_head):
    x = np.asarray(x, dtype=np.float32)
    w_qkv = np.asarray(w_qkv, dtype=np.float32)
    w_swiglu = np.asarray(w_swiglu, dtype=np.float32)
    w_out = np.asarray(w_out, dtype=np.float32)
    B, T, _ = x.shape
    nc = _get_nc(T)
    in_maps = host_prep(x, w_qkv, w_swiglu, w_out, T)
    res = run_bass_kernel_spmd(nc, in_maps, list(range(NCORES)))
    return assemble(res.results, B, T)
